# revision 4
# baseline (speedup 1.0000x reference)
"""GQA causal attention (ternary weights) on 8 TRN2 NeuronCores — v3.

fp16 hi/lo exact compute core (proven precise) inside a fine-grained
interleaved schedule:
  - core c owns Q heads [4c, 4c+4) and KV head c (tensor-parallel per hint).
  - per (batch, 512-token tile): x DMA (fp16 hi+lo) -> q/k/v projections as
    2-pass fp16 matmuls -> S~ row-max pass ([q,k] fp16-hi matmul, causal mask
    of the diagonal block folded in as an fp16 identity-x-mask matmul, DVE
    tensor_reduce) -> exact S^T ([k,q], fp16 hi pass with folded -max bias
    row + compensated fp16 lo pass + fp16 mask matmul) -> exp on ScalarE
    (fp16 out, pipelined one chunk ahead of PV) -> PV + row-sums (fp16
    matmul against V with a ones column) -> o_proj partials (fp16, deferred
    one tile and interleaved) -> fp16 out DMA.
  - four instruction streams (S~, exact, o_proj, projections) are
    round-robin interleaved so the PE never starves behind the DVE-paced
    max pass or the ScalarE-paced exp; the last head's attention is
    deferred into the next tile's first slot.
  - host sums the 8 partial outputs (the all-reduce of the row-split o_proj).
"""

import sys

sys.path.insert(0, "/opt/trn_rl_repo")

import numpy as np

B = 2
S = 2048
D = 2048
NCORES = 8
HEADS_PER_CORE = 4
HD = 64
QROWS = HEADS_PER_CORE * HD  # 256
TT = 512
LO_SCALE = 1024.0
LO8 = 32.0
MASK_NEG = -30000.0
MASK_NEG2 = -60000.0

_CACHE = {}


def _build_program(b=B, s=S, d=D):
    import concourse.bacc as bacc
    import concourse.tile as tile
    import concourse.mybir as mybir
    from concourse import masks
    from contextlib import ExitStack

    f32 = mybir.dt.float32
    f32r = mybir.dt.float32r
    f16 = mybir.dt.float16
    Alu = mybir.AluOpType
    Act = mybir.ActivationFunctionType

    tokens = b * s
    n_dc = d // 128          # contraction chunks for projections (16)
    tt_per_b = s // TT       # 4
    n_qc_t = TT // 128       # q chunks per tile (4)
    n_mt = d // 128          # o_proj output row tiles (16)
    n_oc = QROWS // 128      # o_proj contraction chunks (2)
    n_ch = tokens // 128     # global 128-token chunks (32)

    nc = bacc.Bacc("TRN2", target_bir_lowering=False, debug=False,
                   num_devices=NCORES)

    f8 = mybir.dt.float8e4
    xh_d = nc.dram_tensor("xh", [d, tokens], f16, kind="ExternalInput").ap()
    xl8_d = nc.dram_tensor("xl8", [d, tokens], f8, kind="ExternalInput").ap()
    wqh_d = nc.dram_tensor("wq_hi", [d, QROWS], f16, kind="ExternalInput").ap()
    wq8_d = nc.dram_tensor("wq8", [d, QROWS], f8, kind="ExternalInput").ap()
    wkh_d = nc.dram_tensor("wkv_hi", [d, 128], f16, kind="ExternalInput").ap()
    wkv8_d = nc.dram_tensor("wkv8", [d, 128], f8, kind="ExternalInput").ap()
    wo_d = nc.dram_tensor("wo", [QROWS, d], f16, kind="ExternalInput").ap()
    out_d = nc.dram_tensor("out", [d, tokens], f16, kind="ExternalOutput").ap()

    with tile.TileContext(nc) as tc, ExitStack() as top:
        constp = top.enter_context(tc.tile_pool(name="const", bufs=1))
        wpool = top.enter_context(tc.tile_pool(name="wts", bufs=1))
        pp = top.enter_context(tc.tile_pool(name="persist", bufs=1))

        # --- constants -------------------------------------------------
        # [k,q] diag mask for the exact pass: -3e4 where k>q
        maskM = constp.tile([128, 128], f16, tag="maskM")
        nc.gpsimd.memset(maskM[:], 0.0)
        nc.gpsimd.affine_select(
            out=maskM[:], in_=maskM[:], compare_op=Alu.is_ge,
            fill=MASK_NEG, base=0, pattern=[[1, 128]], channel_multiplier=-1)
        # [q,k] diag mask for the S~ max pass: -6e4 where k>q
        maskM2 = constp.tile([128, 128], f16, tag="maskM2")
        nc.gpsimd.memset(maskM2[:], 0.0)
        nc.gpsimd.affine_select(
            out=maskM2[:], in_=maskM2[:], compare_op=Alu.is_ge,
            fill=MASK_NEG2, base=0, pattern=[[-1, 128]], channel_multiplier=1)
        ident = constp.tile([128, 128], f32, tag="ident")
        masks.make_identity(nc, ident[:])
        ident16 = constp.tile([128, 128], f16, tag="ident16")
        nc.scalar.copy(ident16[:], ident[:])
        onesc = constp.tile([65, HD], f32r, tag="onesc")
        nc.scalar.activation(onesc[:], ident[0:65, 0:HD], Act.Identity,
                             bias=1.0, scale=0.0)

        # --- weights ---------------------------------------------------
        wqh_sb = wpool.tile([128, n_dc * QROWS], f16, tag="wqh", name="wqh")
        nc.scalar.dma_start(
            out=wqh_sb[:].rearrange("p (c n) -> p c n", n=QROWS),
            in_=wqh_d.rearrange("(c p) n -> p c n", p=128))
        wq8_sb = wpool.tile([128, n_dc * QROWS], f8, tag="wq8", name="wq8")
        nc.scalar.dma_start(
            out=wq8_sb[:].rearrange("p (c n) -> p c n", n=QROWS),
            in_=wq8_d.rearrange("(c p) n -> p c n", p=128))
        wkh_sb = wpool.tile([128, n_dc * 128], f16, tag="wkh", name="wkh")
        nc.scalar.dma_start(
            out=wkh_sb[:].rearrange("p (c n) -> p c n", n=128),
            in_=wkh_d.rearrange("(c p) n -> p c n", p=128))
        wkv8_sb = wpool.tile([128, n_dc * 128], f8, tag="wkv8", name="wkv8")
        nc.scalar.dma_start(
            out=wkv8_sb[:].rearrange("p (c n) -> p c n", n=128),
            in_=wkv8_d.rearrange("(c p) n -> p c n", p=128))
        wo_sb = wpool.tile([128, n_oc * d], f16, tag="wo", name="wo_sb")
        nc.scalar.dma_start(
            out=wo_sb[:].rearrange("p (c n) -> p c n", n=d),
            in_=wo_d.rearrange("(c p) n -> p c n", p=128))

        # --- persistent activations -----------------------------------
        # qA[h]: rows 0:64 = fp16(q/8) "qh", row 64 = m~ (row max)
        # qB8[h]: fp8 pairs [64, 2, tokens]: t0 = qh/32, t1 = 32*(q/8 - qh)
        # khb:   rows 0:64 = fp16(k) "kh", row 64 = -1
        # klkh8: fp8 pairs [64, 2, tokens]: t0 = 32*(k - kh), t1 = kh/32
        # vhat:  [128, ch*65]: cols 0:64 = v (token-major), col 64 = 1.0
        qA = [pp.tile([65, tokens], f16, tag=f"qA{h}", name=f"qA{h}")
              for h in range(HEADS_PER_CORE)]
        qB8 = [pp.tile([64, 2 * tokens], f8, tag=f"qB8{h}", name=f"qB8{h}")
               for h in range(HEADS_PER_CORE)]
        khb = pp.tile([65, tokens], f16, tag="khb")
        nc.gpsimd.memset(khb[64:65, :], -1.0)
        klkh8 = pp.tile([64, 2 * tokens], f8, tag="klkh8")
        vhat = pp.tile([128, n_ch * 65], f16, tag="vhat")
        nc.scalar.activation(
            vhat[:], ident[:, 0:1].to_broadcast([128, n_ch * 65]),
            Act.Identity, bias=1.0, scale=0.0)
        ao = [pp.tile([128, tokens], f16, tag=f"ao{i}", name=f"ao{i}")
              for i in range(2)]

        with ExitStack() as ph:
            xp = ph.enter_context(tc.tile_pool(name="xp", bufs=4))
            x8p = ph.enter_context(tc.tile_pool(name="x8p", bufs=2))
            sp = ph.enter_context(tc.tile_pool(name="sp", bufs=3))
            ptp = ph.enter_context(tc.tile_pool(name="ptp", bufs=5))
            osp = ph.enter_context(tc.tile_pool(name="osp", bufs=4))
            mchp = ph.enter_context(tc.tile_pool(name="mchp", bufs=4))
            mrp = ph.enter_context(tc.tile_pool(name="mrp", bufs=4))
            recp = ph.enter_context(tc.tile_pool(name="recp", bufs=2))
            psA = ph.enter_context(
                tc.tile_pool(name="psA", bufs=2, space="PSUM"))
            psS = ph.enter_context(
                tc.tile_pool(name="psS", bufs=4, space="PSUM"))
            psav = ph.enter_context(
                tc.tile_pool(name="psav", bufs=2, space="PSUM"))

            drains = []

            def drain_head(bb, h, qt, pav):
                boff = bb * s
                qlo = boff + qt * TT
                rec = recp.tile([65, TT], f32r, tag="rec", name="rec")
                with nc.allow_low_precision(
                        reason="1/l broadcast feeds fp32r matmul"):
                    nc.vector.reciprocal(rec[64:65, :], pav[64:65, :])
                bct = psS.tile([128, TT], f32, tag="st", name="bct")
                bc = bct[0:64, :]
                nc.tensor.matmul(
                    bc, lhsT=onesc[64:65, 0:HD], rhs=rec[64:65, :],
                    start=True, stop=True)
                bcs = recp.tile([64, TT], f32, tag="bcs", name="bcs")
                nc.scalar.copy(bcs[:], bc)
                rows = slice((h % 2) * 64, (h % 2) * 64 + 64)
                nc.vector.tensor_tensor(
                    ao[h // 2][rows, qlo:qlo + TT], pav[0:64, :], bcs[:],
                    op=Alu.mult)

            def smax_items(bb, ltt, h):
                """Per-item closures: S~ matmul+reduce tiles, then m~ pack."""
                boff = bb * s
                qlo = boff + ltt * TT
                mq = mchp.tile([128, n_qc_t], f32, tag="mq", name="mq")
                items = []
                for qcl in range(n_qc_t):
                    qc = ltt * n_qc_t + qcl
                    ntk = qc // n_qc_t + 1
                    mtm = mchp.tile([128, 8], f32, tag="mtm", name="mtm")
                    for kt in range(ntk):
                        def s_tile(qc=qc, qcl=qcl, kt=kt, ntk=ntk, mtm=mtm):
                            w = min(TT, (qc + 1) * 128 - kt * TT)
                            last = kt == ntk - 1
                            st = psS.tile([128, TT], f32, tag="st", name="st")
                            nc.tensor.matmul(
                                st[:, 0:w],
                                lhsT=qA[h][0:64, boff + qc * 128:
                                           boff + qc * 128 + 128],
                                rhs=khb[0:64, boff + kt * TT:
                                        boff + kt * TT + w],
                                start=True, stop=not last,
                                skip_group_check=True)
                            if last:
                                nc.tensor.matmul(
                                    st[:, w - 128:w], lhsT=ident16[:],
                                    rhs=maskM2[:], start=False, stop=True,
                                    skip_group_check=True)
                            nc.vector.tensor_reduce(
                                mtm[:, kt:kt + 1], st[:, 0:w],
                                axis=mybir.AxisListType.X, op=Alu.max)
                            if last:
                                nc.vector.tensor_reduce(
                                    mq[:, qcl:qcl + 1], mtm[:, 0:ntk],
                                    axis=mybir.AxisListType.X, op=Alu.max)
                        items.append(s_tile)

                def pack():
                    tp = psS.tile([128, TT], f32, tag="st", name="tp")
                    nc.tensor.transpose(tp[0:n_qc_t, 0:128], mq[:, 0:n_qc_t],
                                        ident[:, 0:128])
                    mrow = mrp.tile([n_qc_t, 128], f32, tag="mrow",
                                    name="mrow")
                    nc.vector.tensor_copy(mrow[:], tp[0:n_qc_t, 0:128])
                    nc.gpsimd.dma_start(
                        out=qA[h][64:65, qlo:qlo + TT].rearrange(
                            "o (c t) -> o c t", t=128),
                        in_=mrow[:])
                return items, pack

            def exact_items(bb, ltt, h):
                """Per-chunk closures: hi+lo S^T matmuls, exp, PV."""
                boff = bb * s
                qlo = boff + ltt * TT
                nchunks = (ltt + 1) * n_qc_t
                state = {}

                def chunk(kc, state=state):
                    if kc == 0:
                        state["pav"] = psav.tile([65, TT], f32, tag="pav",
                                                 name="pav")
                        state["q"] = []
                    pav = state["pav"]
                    j = kc - ltt * n_qc_t
                    lo = max(j, 0) * 128
                    ksl = slice(boff + kc * 128, boff + kc * 128 + 128)
                    s2 = psS.tile([128, TT], f32, tag="st", name="s2")
                    nc.tensor.matmul(
                        s2[:, lo:TT], lhsT=khb[:, ksl],
                        rhs=qA[h][:, qlo + lo:qlo + TT],
                        start=True, stop=False, skip_group_check=True)
                    nc.tensor.matmul(
                        s2[:, lo:TT],
                        lhsT=klkh8[:].rearrange(
                            "p (t n) -> p t n", t=2)[:, :, ksl],
                        rhs=qB8[h][:].rearrange(
                            "p (t n) -> p t n", t=2)[:, :,
                                                     qlo + lo:qlo + TT],
                        start=False, stop=j < 0,
                        perf_mode=mybir.MatmulPerfMode.DoubleRow,
                        skip_group_check=True)
                    if j >= 0:
                        nc.tensor.matmul(
                            s2[:, lo:lo + 128], lhsT=ident16[:], rhs=maskM[:],
                            start=False, stop=True, skip_group_check=True)
                    pt = ptp.tile([128, TT], f16, tag="pt", name="pt")
                    nc.scalar.activation(pt[:, lo:TT], s2[:, lo:TT], Act.Exp)
                    state["q"].append((kc, pt, lo))

                    def pv():
                        pkc, ppt, plo = state["q"].pop(0)
                        ch = bb * (s // 128) + pkc
                        nc.tensor.matmul(
                            pav[:, plo:TT],
                            lhsT=vhat[:, ch * 65:ch * 65 + 65],
                            rhs=ppt[:, plo:TT],
                            start=(pkc == 0), stop=(pkc == nchunks - 1),
                            skip_group_check=True)
                    if len(state["q"]) > 2:
                        pv()
                    if kc == nchunks - 1:
                        while state["q"]:
                            pv()
                        if drains:
                            drains.pop(0)()
                        drains.append(
                            lambda: drain_head(bb, h, ltt, pav))
                        if h == HEADS_PER_CORE - 1:
                            while drains:
                                drains.pop(0)()
                return [lambda kc=kc: chunk(kc) for kc in range(nchunks)]

            def oproj_items(bb, ltt):
                qlo = bb * s + ltt * TT

                def otile(m):
                    po = psS.tile([128, TT], f32, tag="st", name="po")
                    for ci in range(n_oc):
                        nc.tensor.matmul(
                            po[:],
                            lhsT=wo_sb[:, ci * d + m * 128:
                                       ci * d + m * 128 + 128],
                            rhs=ao[ci][:, qlo:qlo + TT],
                            start=(ci == 0), stop=(ci == n_oc - 1))
                    osb = osp.tile([128, TT], f16, tag="osb", name="osb")
                    if m % 2 == 0:
                        nc.scalar.copy(osb[:], po[:])
                    else:
                        nc.vector.tensor_copy(osb[:], po[:])
                    nc.sync.dma_start(
                        out=out_d[m * 128:(m + 1) * 128, qlo:qlo + TT],
                        in_=osb[:])
                return [lambda m=m: otile(m) for m in range(n_mt)]

            def interleave(*streams):
                streams = [list(it) for it in streams if it]
                n = max((len(st) for st in streams), default=0)
                for i in range(n):
                    for st in streams:
                        lo = i * len(st) // n
                        hi = (i + 1) * len(st) // n
                        for fn in st[lo:hi]:
                            fn()

            carry = None   # deferred exact items of the previous tile's h3
            carry_op = []  # deferred o_proj items of the previous tile
            for bb in range(b):
                boff = bb * s
                for ltt in range(tt_per_b):
                    tcols = slice(boff + ltt * TT, boff + (ltt + 1) * TT)
                    # ---- x DMA (fp16 hi halves + one fp8 lo tile) ----
                    xh = []
                    for half in range(2):
                        t = xp.tile([128, (n_dc // 2) * TT], f16,
                                    tag="x", name="xtile")
                        nc.sync.dma_start(
                            out=t[:].rearrange("p (c t) -> p c t", t=TT),
                            in_=xh_d.rearrange("(c p) t -> p c t", p=128)[
                                :, half * 8:half * 8 + 8, tcols])
                        xh.append(t)
                    x8 = x8p.tile([128, n_dc * TT], f8, tag="x8", name="x8t")
                    nc.sync.dma_start(
                        out=x8[:].rearrange("p (c t) -> p c t", t=TT),
                        in_=xl8_d.rearrange("(c p) t -> p c t", p=128)[
                            :, :, tcols])

                    # ---- projections (fp16 hi + fp8 DoubleRow lo) ----
                    def proj(w16, w8, mcol, mwid, ps):
                        for half in range(2):
                            for c in range(8):
                                cg = half * 8 + c
                                nc.tensor.matmul(
                                    ps[:],
                                    lhsT=w16[:, cg * mwid + mcol:
                                             cg * mwid + mcol + 128],
                                    rhs=xh[half][:, c * TT:(c + 1) * TT],
                                    start=(cg == 0), stop=False,
                                    skip_group_check=True)
                        w8r = w8[:].rearrange("p (c n) -> p c n", n=mwid)
                        x8r = x8[:].rearrange("p (c t) -> p c t", t=TT)
                        for pr in range(n_dc // 2):
                            nc.tensor.matmul(
                                ps[:],
                                lhsT=w8r[:, 2 * pr:2 * pr + 2,
                                         mcol:mcol + 128],
                                rhs=x8r[:, 2 * pr:2 * pr + 2, :],
                                start=False, stop=(pr == n_dc // 2 - 1),
                                perf_mode=mybir.MatmulPerfMode.DoubleRow,
                                skip_group_check=True)

                    for m in range(2):
                        ps = psA.tile([128, TT], f32, tag="ps", name="ps")
                        proj(wqh_sb, wq8_sb, m * 128, QROWS, ps)
                        for i in range(2):
                            h = 2 * m + i
                            rows = slice(i * 64, i * 64 + 64)
                            nc.scalar.mul(qA[h][0:64, tcols], ps[rows, :],
                                          0.125)
                            qB8r = qB8[h][:].rearrange("p (t n) -> p t n",
                                                       t=2)
                            nc.gpsimd.tensor_scalar_mul(
                                qB8r[:, 0, tcols], qA[h][0:64, tcols],
                                1.0 / LO8)
                            res = sp.tile([64, TT], f32, tag="res",
                                          name="res")
                            nc.vector.scalar_tensor_tensor(
                                res[:], in0=ps[rows, :], scalar=0.125,
                                in1=qA[h][0:64, tcols], op0=Alu.mult,
                                op1=Alu.subtract)
                            nc.gpsimd.tensor_scalar_mul(
                                qB8r[:, 1, tcols], res[:], LO8)
                    ps = psA.tile([128, TT], f32, tag="ps", name="ps")
                    proj(wkh_sb, wkv8_sb, 0, 128, ps)
                    nc.scalar.copy(khb[0:64, tcols], ps[0:64, :])
                    klkh8r = klkh8[:].rearrange("p (t n) -> p t n", t=2)
                    nc.gpsimd.tensor_scalar_mul(
                        klkh8r[:, 1, tcols], khb[0:64, tcols], 1.0 / LO8)
                    res = sp.tile([64, TT], f32, tag="res", name="res")
                    nc.vector.scalar_tensor_tensor(
                        res[:], in0=ps[0:64, :], scalar=1.0,
                        in1=khb[0:64, tcols], op0=Alu.mult, op1=Alu.subtract)
                    nc.gpsimd.tensor_scalar_mul(
                        klkh8r[:, 0, tcols], res[:], LO8)
                    vtmp = sp.tile([64, TT], f32, tag="vtmp", name="vtmp")
                    nc.scalar.copy(vtmp[:], ps[64:128, :])
                    for j in range(n_qc_t):
                        ptr = psS.tile([128, TT], f32, tag="st", name="ptr")
                        nc.tensor.transpose(ptr[0:128, 0:64],
                                            vtmp[:, j * 128:(j + 1) * 128],
                                            ident[0:64, 0:64])
                        ch = (bb * tt_per_b + ltt) * n_qc_t + j
                        nc.scalar.copy(vhat[:, ch * 65:ch * 65 + 64],
                                       ptr[0:128, 0:64])

                    # ---- interleaved slots ---------------------------
                    for h in range(HEADS_PER_CORE):
                        sitems, pack = smax_items(bb, ltt, h)
                        if h == 0:
                            interleave(carry or [], sitems)
                            interleave(carry_op[0:4])
                        else:
                            # emit a few S~ tiles first so the PE queue is
                            # not headed by an exact chunk still waiting on
                            # the previous head's m~ scatter DMA
                            for fn in sitems[0:3]:
                                fn()
                            interleave(sitems[3:],
                                       exact_items(bb, ltt, h - 1),
                                       carry_op[4 * h:4 * h + 4])
                        pack()
                    carry = exact_items(bb, ltt, HEADS_PER_CORE - 1)
                    carry_op = oproj_items(bb, ltt)

            # tail: last tile's h3 + drains + its o_proj
            interleave(carry)
            while drains:
                drains.pop(0)()
            interleave(carry_op)

    nc.compile()
    return nc


def _ternarize(w):
    w = np.asarray(w, np.float32)
    scale = max(np.abs(w).mean(), 1e-6)
    return ((w > 0.05 * scale).astype(np.float32)
            - (w < -0.05 * scale).astype(np.float32))


def _split_f16(a):
    hi = a.astype(np.float16)
    lo = ((a - hi.astype(np.float32)) * LO_SCALE).astype(np.float16)
    return hi, lo


def kernel(x, wq, wk, wv, wo):
    from concourse.bass_utils import run_bass_kernel_spmd

    if "nc" not in _CACHE:
        _CACHE["nc"] = _build_program()
    nc = _CACHE["nc"]

    tq = _ternarize(wq)
    tk = _ternarize(wk)
    tv = _ternarize(wv)
    to = _ternarize(wo)

    import ml_dtypes
    f8t = ml_dtypes.float8_e4m3fn

    xT = np.ascontiguousarray(np.asarray(x, np.float32).reshape(B * S, D).T)
    xh = xT.astype(np.float16)
    xl8 = ((xT - xh.astype(np.float32)) * LO8).astype(f8t)

    in_maps = []
    for c in range(NCORES):
        qsl = slice(c * QROWS, (c + 1) * QROWS)
        ksl = slice(c * HD, (c + 1) * HD)
        wkv = np.concatenate([tk[ksl], tv[ksl]], axis=0)  # [128, D]
        in_maps.append({
            "xh": xh, "xl8": xl8,
            "wq_hi": np.ascontiguousarray(tq[qsl].T).astype(np.float16),
            "wq8": np.ascontiguousarray(tq[qsl].T / LO8).astype(f8t),
            "wkv_hi": np.ascontiguousarray(wkv.T).astype(np.float16),
            "wkv8": np.ascontiguousarray(wkv.T / LO8).astype(f8t),
            "wo": np.ascontiguousarray(to[:, qsl].T).astype(np.float16),
        })

    res = run_bass_kernel_spmd(nc, in_maps, list(range(NCORES)))
    total = res.results[0]["out"].astype(np.float32)
    for c in range(1, NCORES):
        total += res.results[c]["out"]
    return np.ascontiguousarray(total.T).reshape(B, S, D).astype(np.float32)
